# revision 1
# baseline (speedup 1.0000x reference)
"""Contrastive loss (NT-Xent) on 8 Trainium2 NeuronCores.

Row-parallel over the [2B, 2B] similarity matrix: core c computes rows
[c*1024, (c+1)*1024). Inputs are passed host-transposed ([D, 2B]) with the
column blocks rotated per core so the diagonal / positive blocks land at
fixed tile indices on every core (uniform SPMD program). Matmuls run in bf16
(full-rate PE path that engages the HAM clock un-throttle). Row-wise logsumexp uses the ACT
engine's fused accumulate; positives and the diagonal exclusion use
identity-mask reduces. Per-core partial sums are combined with a scalar
AllGather.
"""

import os
import sys

for _p in ("/opt/trn_rl_repo", "/root/.axon_site/_ro/trn_rl_repo"):
    if os.path.isdir(_p) and _p not in sys.path:
        sys.path.append(_p)

import numpy as np

B = 4096
D = 1024
TWO_B = 2 * B
TEMP = 0.07
N_CORES = 8
BLK = TWO_B // N_CORES  # 1024 rows per core
NT = TWO_B // 512  # 16 column tiles of 512
MT = BLK // 128  # 8 row tiles of 128
KT = D // 128  # 8 contraction chunks of 128

_cache = {}


def _build():
    import concourse.bass as bass
    import concourse.bacc as bacc
    import concourse.mybir as mybir
    from concourse.tile import TileContext

    f32 = mybir.dt.float32
    f32r = mybir.dt.float32r
    bf16 = mybir.dt.bfloat16
    AF = mybir.ActivationFunctionType
    ALU = mybir.AluOpType
    AX = mybir.AxisListType

    nc = bacc.Bacc(None, target_bir_lowering=False, debug=False)
    ft = nc.dram_tensor("ft", [D, TWO_B], f32, kind="ExternalInput")
    perm = nc.dram_tensor("perm", [8, 8], f32, kind="ExternalInput")
    ident = nc.dram_tensor("ident", [128, 128], f32, kind="ExternalInput")
    maskinv = nc.dram_tensor("maskinv", [128, 128], f32, kind="ExternalInput")
    loss = nc.dram_tensor("loss", [1, 1], f32, kind="ExternalOutput")

    with TileContext(nc) as tc:
        with (
            tc.tile_pool(name="own", bufs=KT) as pool_own,
            tc.tile_pool(name="big", bufs=1) as pool_big,
            tc.tile_pool(name="sq", bufs=2) as pool_sq,
            tc.tile_pool(name="rhs", bufs=10) as pool_rhs,
            tc.tile_pool(name="rhsr", bufs=10) as pool_rhsr,
            tc.tile_pool(name="exp", bufs=4) as pool_exp,
            tc.tile_pool(name="small", bufs=1) as pool_small,
            tc.tile_pool(name="rot", bufs=1) as pool_rot,
            tc.tile_pool(name="junk", bufs=2) as pool_junk,
            tc.tile_pool(name="psum", bufs=8, space="PSUM") as psum,
            tc.tile_pool(name="dram", bufs=4, space="DRAM") as dram,
        ):
            warm_in = dram.tile([1, 1], f32, name="warm_in")
            warm_out = dram.tile([8, 1], f32, name="warm_out")
            inv_cc_in = dram.tile([1, BLK], f32, name="inv_cc_in")
            inv_cc_out = dram.tile([8, BLK], f32, name="inv_cc_out")
            part_in = dram.tile([1, 1], f32, name="part_in")
            part_out = dram.tile([8, 1], f32, name="part_out")

            # --- collective-stack warmup: absorbs one-time ncfw/channel setup
            # concurrently with the prologue ---
            warm_sb = pool_small.tile([1, 1], f32, name="warm_sb", tag="warm_sb")
            nc.vector.memset(warm_sb[:], 0.0)
            nc.sync.dma_start(out=warm_in[:], in_=warm_sb[:])
            nc.gpsimd.collective_compute(
                "AllGather",
                mybir.AluOpType.bypass,
                ins=[warm_in.opt()],
                outs=[warm_out.opt()],
                replica_groups=[list(range(N_CORES))],
            )

            # --- constants ---
            ones_f = pool_small.tile([128, 1], f32, name="ones_f", tag="ones_f")
            nc.vector.memset(ones_f[:], 1.0)
            ones_r = pool_small.tile([128, 1], bf16, name="ones_r", tag="ones_r")
            nc.vector.tensor_copy(ones_r[:], ones_f[:])
            ones1_f = pool_small.tile([1, 128], f32, name="ones1_f", tag="ones1_f")
            nc.vector.memset(ones1_f[:], 1.0)
            ones1_r = pool_small.tile([1, 128], bf16, name="ones1_r", tag="ones1_r")
            nc.vector.tensor_copy(ones1_r[:], ones1_f[:])
            ident_sb = pool_small.tile([128, 128], f32, name="ident", tag="ident")
            nc.sync.dma_start(out=ident_sb[:], in_=ident[:])
            maskinv_sb = pool_small.tile([128, 128], f32, name="maskinv", tag="maskinv")
            nc.sync.dma_start(out=maskinv_sb[:], in_=maskinv[:])
            perm_f = pool_small.tile([8, 8], f32, name="perm_f", tag="perm_f")
            nc.sync.dma_start(out=perm_f[:], in_=perm[:])
            perm_r = pool_small.tile([8, 8], bf16, name="perm_r", tag="perm_r")
            nc.vector.tensor_copy(perm_r[:], perm_f[:])

            # --- own block: load + row norms ---
            own_raw = []
            for k in range(KT):
                t = pool_own.tile([128, BLK], f32, name="own_raw", tag="own_raw")
                nc.sync.dma_start(
                    out=t[:], in_=ft[k * 128 : (k + 1) * 128, 0:BLK]
                )
                own_raw.append(t)

            pss = [psum.tile([128, 512], f32, name="ps", tag="ps") for _ in range(2)]
            for k in range(KT):
                s = pool_sq.tile([128, BLK], bf16, name="sq", tag="sq")
                nc.vector.tensor_mul(s[:], own_raw[k][:], own_raw[k][:])
                for h in range(2):
                    nc.tensor.matmul(
                        pss[h][0:1, :],
                        ones_r[:],
                        s[:, h * 512 : (h + 1) * 512],
                        start=(k == 0),
                        stop=(k == KT - 1),
                    )
            nrm = pool_small.tile([1, BLK], f32, name="nrm", tag="nrm")
            for h in range(2):
                nc.scalar.activation(
                    nrm[:, h * 512 : (h + 1) * 512], pss[h][0:1, :], AF.Sqrt
                )
            inv_own = pool_small.tile([1, BLK], f32, name="inv_own", tag="inv_own")
            nc.vector.reciprocal(inv_own[:], nrm[:])

            # share inverse norms across cores
            nc.sync.dma_start(out=inv_cc_in[:], in_=inv_own[:])
            nc.gpsimd.collective_compute(
                "AllGather",
                mybir.AluOpType.bypass,
                ins=[inv_cc_in.opt()],
                outs=[inv_cc_out.opt()],
                replica_groups=[list(range(N_CORES))],
            )

            # binv[:, j*1024 + q] = inv norm of rotated column block j, col q,
            # replicated across all 128 partitions (PE rank-1 broadcast).
            binv = pool_big.tile([128, TWO_B], f32, name="binv", tag="binv")
            inv_own_r = pool_small.tile([1, BLK], bf16, name="inv_own_r", tag="inv_own_r")
            nc.vector.tensor_copy(inv_own_r[:], inv_own[:])
            for h in range(2):
                pb = psum.tile([128, 512], f32, name="ps", tag="ps")
                nc.tensor.matmul(
                    pb[:],
                    ones1_r[:],
                    inv_own_r[0:1, h * 512 : (h + 1) * 512],
                    start=True,
                    stop=True,
                )
                nc.vector.tensor_copy(binv[:, h * 512 : (h + 1) * 512], pb[:])

            # own block normalized: lhsT for all matmuls, rhs for n in {0, 1}
            own_nrm = []
            for k in range(KT):
                t = pool_own.tile([128, BLK], bf16, name="own_nrm", tag="own_nrm")
                nc.vector.tensor_mul(t[:], own_raw[k][:], binv[:, 0:BLK])
                own_nrm.append(t[:])

            # rotated inverse norms of the remote blocks
            g_inv = pool_small.tile([8, BLK], f32, name="g_inv", tag="g_inv")
            nc.sync.dma_start(out=g_inv[:], in_=inv_cc_out[:])
            g_inv_r = pool_small.tile([8, BLK], bf16, name="g_inv_r", tag="g_inv_r")
            nc.vector.tensor_copy(g_inv_r[:], g_inv[:])
            rot_r = pool_small.tile([8, BLK], bf16, name="rot_r", tag="rot_r")
            for h in range(2):
                pr = psum.tile([128, 512], f32, name="ps", tag="ps")
                nc.tensor.matmul(
                    pr[0:8, :],
                    perm_r[:],
                    g_inv_r[:, h * 512 : (h + 1) * 512],
                    start=True,
                    stop=True,
                )
                nc.vector.tensor_copy(rot_r[:, h * 512 : (h + 1) * 512], pr[0:8, :])
            # PE operands must start at partition 0/32/64 — move each rotated
            # row onto partition 0 before its rank-1 broadcast.
            for j in range(1, 8):
                rf = pool_rot.tile([1, BLK], bf16, name="rf", tag="rf")
                nc.sync.dma_start(out=rf[:], in_=rot_r[j : j + 1, :])
                for h in range(2):
                    pb = psum.tile([128, 512], f32, name="ps", tag="ps")
                    nc.tensor.matmul(
                        pb[:],
                        ones1_r[:],
                        rf[0:1, h * 512 : (h + 1) * 512],
                        start=True,
                        stop=True,
                    )
                    nc.vector.tensor_copy(
                        binv[:, j * BLK + h * 512 : j * BLK + (h + 1) * 512], pb[:]
                    )

            # --- accumulators ---
            rs_buf = pool_big.tile([128, MT * NT], f32, name="rs_buf", tag="rs_buf")
            pos_all = pool_small.tile([128, MT], f32, name="pos_all", tag="pos_all")
            nc.vector.memset(pos_all[:], 0.0)

            # --- main loop: one 512-wide column tile at a time ---
            n_limit = int(os.environ.get("CL_NT", NT))
            for n in range(n_limit):
                if n < 2:
                    rhs = [own_nrm[k][:, n * 512 : (n + 1) * 512] for k in range(KT)]
                else:
                    rhs = []
                    for k in range(KT):
                        raw = pool_rhs.tile([128, 512], f32, name="rhs_raw", tag="rhs_raw")
                        nc.sync.dma_start(
                            out=raw[:],
                            in_=ft[k * 128 : (k + 1) * 128, n * 512 : (n + 1) * 512],
                        )
                        r = pool_rhsr.tile([128, 512], bf16, name="rhs_r", tag="rhs_r")
                        nc.vector.tensor_mul(
                            r[:], raw[:], binv[:, n * 512 : (n + 1) * 512]
                        )
                        rhs.append(r[:])
                for m in range(MT):
                    ps = psum.tile([128, 512], f32, name="ps", tag="ps")
                    for k in range(KT):
                        nc.tensor.matmul(
                            ps[:],
                            own_nrm[k][:, m * 128 : (m + 1) * 128],
                            rhs[k],
                            start=(k == 0),
                            stop=(k == KT - 1),
                        )
                    sl = (m % 4) * 128
                    if n == 8 + m // 4:
                        # positives: diagonal of this 128x128 slab (raw sim)
                        junk = pool_junk.tile([128, 128], f32, name="junk", tag="junk")
                        nc.vector.tensor_mul(junk[:], ps[:, sl : sl + 128], ident_sb[:])
                        nc.vector.reduce_sum(
                            out=pos_all[:, m : m + 1], in_=junk[:], axis=AX.X
                        )
                    if n == m // 4:
                        # diagonal block: exp, zero the self-sim, reduce on DVE
                        e = pool_exp.tile([128, 512], f32, name="exp", tag="exp")
                        nc.scalar.activation(e[:], ps[:], AF.Exp, scale=1.0 / TEMP)
                        nc.vector.tensor_mul(
                            e[:, sl : sl + 128], e[:, sl : sl + 128], maskinv_sb[:]
                        )
                        nc.vector.reduce_sum(
                            out=rs_buf[:, m * NT + n : m * NT + n + 1],
                            in_=e[:],
                            axis=AX.X,
                        )
                    else:
                        e = pool_exp.tile([128, 512], f32, name="exp", tag="exp")
                        nc.scalar.activation(
                            e[:],
                            ps[:],
                            AF.Exp,
                            scale=1.0 / TEMP,
                            accum_out=rs_buf[:, m * NT + n : m * NT + n + 1],
                        )

            # --- logsumexp + loss ---
            rs_all = pool_small.tile([128, MT], f32, name="rs_all", tag="rs_all")
            for m in range(MT):
                nc.vector.reduce_sum(
                    out=rs_all[:, m : m + 1],
                    in_=rs_buf[:, m * NT : m * NT + n_limit],
                    axis=AX.X,
                )
            lse = pool_small.tile([128, MT], f32, name="lse", tag="lse")
            nc.scalar.activation(lse[:], rs_all[:], AF.Ln)
            poss = pool_small.tile([128, MT], f32, name="poss", tag="poss")
            nc.vector.tensor_scalar_mul(poss[:], pos_all[:], 1.0 / TEMP)
            diff = pool_small.tile([128, MT], f32, name="diff", tag="diff")
            nc.vector.tensor_sub(diff[:], lse[:], poss[:])
            dsum = pool_small.tile([128, 1], f32, name="dsum", tag="dsum")
            nc.vector.reduce_sum(out=dsum[:], in_=diff[:], axis=AX.X)
            pf = psum.tile([128, 512], f32, name="ps", tag="ps")
            nc.tensor.matmul(
                pf[0:1, 0:1], dsum[:], ones_f[:], start=True, stop=True
            )
            part_sb = pool_small.tile([1, 1], f32, name="part_sb", tag="part_sb")
            nc.vector.tensor_copy(part_sb[:], pf[0:1, 0:1])
            nc.sync.dma_start(out=part_in[:], in_=part_sb[:])
            nc.gpsimd.collective_compute(
                "AllGather",
                mybir.AluOpType.bypass,
                ins=[part_in.opt()],
                outs=[part_out.opt()],
                replica_groups=[list(range(N_CORES))],
            )
            back = pool_small.tile([1, 8], f32, name="back", tag="back")
            nc.sync.dma_start(
                out=back[:], in_=part_out[:].rearrange("a b -> (a b)")[None, :]
            )
            tot = pool_small.tile([1, 1], f32, name="tot", tag="tot")
            nc.vector.reduce_sum(out=tot[:], in_=back[:], axis=AX.X)
            lout = pool_small.tile([1, 1], f32, name="lout", tag="lout")
            nc.scalar.mul(lout[:], tot[:], 1.0 / TWO_B)
            nc.sync.dma_start(out=loss[:], in_=lout[:])

    nc.compile()
    return nc


def kernel(features_1: np.ndarray, features_2: np.ndarray) -> np.ndarray:
    from concourse.bass_utils import run_bass_kernel_spmd

    if "nc" not in _cache:
        _cache["nc"] = _build()
    nc = _cache["nc"]

    f1 = np.ascontiguousarray(np.asarray(features_1, dtype=np.float32))
    f2 = np.ascontiguousarray(np.asarray(features_2, dtype=np.float32))
    f = np.concatenate([f1, f2], axis=0)  # [2B, D]
    ftb = np.ascontiguousarray(f.T).reshape(D, N_CORES, BLK)  # [D, 8, 1024]

    ident = np.eye(128, dtype=np.float32)
    maskinv = (1.0 - ident).astype(np.float32)

    in_maps = []
    for c in range(N_CORES):
        order = [(c + j) % N_CORES for j in range(N_CORES)]
        ft_c = np.ascontiguousarray(ftb[:, order, :]).reshape(D, TWO_B)
        perm_c = np.zeros((8, 8), dtype=np.float32)
        for j in range(N_CORES):
            perm_c[(c + j) % N_CORES, j] = 1.0
        in_maps.append(
            {"ft": ft_c, "perm": perm_c, "ident": ident, "maskinv": maskinv}
        )

    res = run_bass_kernel_spmd(nc, in_maps, list(range(N_CORES)))
    out = res.results[0]["loss"]
    return np.float32(out.reshape(()))



# revision 8
# speedup vs baseline: 1.4809x; 1.4809x over previous
"""Contrastive loss (NT-Xent) on 8 Trainium2 NeuronCores.

Row-parallel over the [2B, 2B] similarity matrix: core c computes rows
[c*1024, (c+1)*1024). Inputs are passed host-transposed ([D, 2B]) in bf16 with
the column blocks rotated per core so diagonal / positive blocks land at fixed
tile indices on every core (uniform SPMD program).

Matmuls run in fp8e4 DoubleRow mode (2 MACs/cell/cycle): contraction pairs are
packed as [128, 2, X] tiles, halving the matmul instruction count vs bf16.
Row norms are computed on the own block, inverse norms shared with a small
AllGather. While that collective (and the one-time CC barrier in front of it)
is in flight, "bridge" column blocks are matmul'd against UNnormalized raw
columns and stashed to SBUF; the per-column inverse-norm scale is applied
after the AllGather lands (scale commutes out of the matmul). Row-wise
logsumexp uses the ACT engine's fused accumulate over 2-PSUM-bank [128,1024]
tiles. Per-core partial sums are combined with a scalar AllGather.
"""

import os
import sys

for _p in ("/opt/trn_rl_repo", "/root/.axon_site/_ro/trn_rl_repo"):
    if os.path.isdir(_p) and _p not in sys.path:
        sys.path.append(_p)

import numpy as np

B = 4096
D = 1024
TWO_B = 2 * B
TEMP = 0.07
N_CORES = 8
BLK = TWO_B // N_CORES  # 1024 columns per block
KP = D // 256  # 4 contraction pair-chunks of 2x128
NT2 = N_CORES  # 8 double-column tiles of 1024 == rotated block index
MT = BLK // 128  # 8 row slabs of 128
FSC = 8.0  # feature pre-scale folded into inv norms (fp8 range use)
PSCALE = 1.0 / (FSC * FSC * TEMP)  # psum -> logits

# Rotated blocks handled as raw-matmul "bridge" tiles while the inv-norm
# AllGather is in flight. Must not contain 0 (own/diag) or 4 (positives).
N_BRIDGE = int(os.environ.get("CL_BRIDGE", 2))
BRIDGE = set(range(1, 1 + N_BRIDGE))

_cache = {}


def _build():
    import concourse.bass as bass
    import concourse.bacc as bacc
    import concourse.mybir as mybir
    from concourse.tile import TileContext

    f32 = mybir.dt.float32
    bf16 = mybir.dt.bfloat16
    f8 = mybir.dt.float8e4
    AF = mybir.ActivationFunctionType
    ALU = mybir.AluOpType
    AX = mybir.AxisListType
    DR = mybir.MatmulPerfMode.DoubleRow

    nc = bacc.Bacc(None, target_bir_lowering=False, debug=False)
    ft = nc.dram_tensor("ft", [D, TWO_B], bf16, kind="ExternalInput")
    bsel = nc.dram_tensor("bsel", [8, 8 * 128], bf16, kind="ExternalInput")
    ident = nc.dram_tensor("ident", [128, 128], f32, kind="ExternalInput")
    maskinv = nc.dram_tensor("maskinv", [128, 128], bf16, kind="ExternalInput")
    warm = nc.dram_tensor("warm", [1, 1], f32, kind="ExternalInput")
    loss = nc.dram_tensor("loss", [1, 1], f32, kind="ExternalOutput")

    def pair_src(kk, c0, w):
        # DRAM view [128, 2, w]: [p, j, c] = ft[256*kk + 128*j + p, c0 + c]
        return ft[256 * kk : 256 * (kk + 1), c0 : c0 + w].rearrange(
            "(j p) c -> p j c", j=2
        )

    with TileContext(nc) as tc:
        with (
            tc.tile_pool(name="own", bufs=KP) as pool_own,
            tc.tile_pool(name="sq", bufs=2) as pool_sq,
            tc.tile_pool(name="nrm8", bufs=KP) as pool_nrm8,
            tc.tile_pool(name="binv", bufs=8) as pool_binv,
            tc.tile_pool(name="raw", bufs=10) as pool_raw,
            tc.tile_pool(name="rhs8", bufs=10) as pool_rhs8,
            tc.tile_pool(name="sim", bufs=8 * len(BRIDGE) if BRIDGE else 1) as pool_sim,
            tc.tile_pool(name="exp", bufs=6) as pool_e,
            tc.tile_pool(name="small", bufs=1) as pool_small,
            tc.tile_pool(name="junk", bufs=2) as pool_junk,
            tc.tile_pool(name="ps2", bufs=3, space="PSUM") as psum2,
            tc.tile_pool(name="ps1", bufs=2, space="PSUM") as psum1,
            tc.tile_pool(name="dram", bufs=4, space="DRAM") as dram,
        ):
            warm_in = dram.tile([1, 1], f32, name="warm_in")
            warm_out = dram.tile([8, 1], f32, name="warm_out")
            inv_in = dram.tile([1, BLK], f32, name="inv_in")
            inv_out = dram.tile([8, BLK], f32, name="inv_out")
            part_in = dram.tile([1, 1], f32, name="part_in")
            part_out = dram.tile([8, 1], f32, name="part_out")

            # --- collective-stack warmup: one DRAM->DRAM DMA dep only, fires
            # immediately so the one-time CC barrier/channel setup overlaps
            # the prologue ---
            nc.sync.dma_start(out=warm_in[:], in_=warm[0:1, 0:1])
            nc.gpsimd.collective_compute(
                "AllGather",
                mybir.AluOpType.bypass,
                ins=[warm_in.opt()],
                outs=[warm_out.opt()],
                replica_groups=[list(range(N_CORES))],
            )

            # --- constants ---
            ones_f = pool_small.tile([128, 1], f32, name="ones_f", tag="ones_f")
            nc.vector.memset(ones_f[:], 1.0)
            ones_r = pool_small.tile([128, 1], bf16, name="ones_r", tag="ones_r")
            nc.vector.tensor_copy(ones_r[:], ones_f[:])
            ones1_f = pool_small.tile([1, 128], f32, name="ones1_f", tag="ones1_f")
            nc.vector.memset(ones1_f[:], 1.0)
            ones1_r = pool_small.tile([1, 128], bf16, name="ones1_r", tag="ones1_r")
            nc.vector.tensor_copy(ones1_r[:], ones1_f[:])
            ident_sb = pool_small.tile([128, 128], f32, name="ident", tag="ident")
            nc.sync.dma_start(out=ident_sb[:], in_=ident[:])
            maskinv_sb = pool_small.tile([128, 128], bf16, name="maskinv", tag="maskinv")
            nc.sync.dma_start(out=maskinv_sb[:], in_=maskinv[:])
            bsel_sb = pool_small.tile([8, 8 * 128], bf16, name="bsel", tag="bsel")
            nc.sync.dma_start(out=bsel_sb[:], in_=bsel[:])

            # --- own block: load pair tiles + row norms ---
            own_raw = []
            for kk in range(KP):
                t = pool_own.tile([128, 2, BLK], bf16, name="own_raw", tag="own_raw")
                nc.sync.dma_start(out=t[:], in_=pair_src(kk, 0, BLK))
                own_raw.append(t)

            pss = [
                psum1.tile([128, 512], f32, name="ps1", tag="ps1") for _ in range(2)
            ]
            for kk in range(KP):
                s = pool_sq.tile([128, 2, BLK], bf16, name="sq", tag="sq")
                nc.vector.tensor_mul(s[:], own_raw[kk][:], own_raw[kk][:])
                for j in range(2):
                    for h in range(2):
                        nc.tensor.matmul(
                            pss[h][0:1, :],
                            ones_r[:],
                            s[:, j, h * 512 : (h + 1) * 512],
                            start=(kk == 0 and j == 0),
                            stop=(kk == KP - 1 and j == 1),
                        )
            nrm = pool_small.tile([1, BLK], f32, name="nrm", tag="nrm")
            for h in range(2):
                # sqrt(ss/64) = norm/8  ->  reciprocal = 8/norm
                nc.scalar.activation(
                    nrm[:, h * 512 : (h + 1) * 512],
                    pss[h][0:1, :],
                    AF.Sqrt,
                    scale=1.0 / (FSC * FSC),
                )
            inv_own = pool_small.tile([1, BLK], f32, name="inv_own", tag="inv_own")
            nc.vector.reciprocal(inv_own[:], nrm[:])

            # share inverse norms across cores
            nc.sync.dma_start(out=inv_in[:], in_=inv_own[:])
            nc.gpsimd.collective_compute(
                "AllGather",
                mybir.AluOpType.bypass,
                ins=[inv_in.opt()],
                outs=[inv_out.opt()],
                replica_groups=[list(range(N_CORES))],
            )

            # broadcast own inv norms to 128 partitions (rank-1 PE trick)
            inv_own_r = pool_small.tile([1, BLK], bf16, name="inv_own_r", tag="inv_own_r")
            nc.vector.tensor_copy(inv_own_r[:], inv_own[:])
            binv_own = pool_binv.tile([128, BLK], bf16, name="binv_own", tag="binv_own")
            for h in range(2):
                pb = psum1.tile([128, 512], f32, name="ps1", tag="ps1")
                nc.tensor.matmul(
                    pb[:],
                    ones1_r[:],
                    inv_own_r[0:1, h * 512 : (h + 1) * 512],
                    start=True,
                    stop=True,
                )
                nc.vector.tensor_copy(binv_own[:, h * 512 : (h + 1) * 512], pb[:])

            # own block normalized to fp8 (x8): lhsT for all matmuls, rhs for np2=0
            own_nrm = []
            for kk in range(KP):
                t = pool_nrm8.tile([128, 2, BLK], f8, name="own_nrm", tag="own_nrm")
                for j in range(2):
                    nc.vector.tensor_mul(t[:, j, :], own_raw[kk][:, j, :], binv_own[:])
                own_nrm.append(t)

            # --- accumulators ---
            rs_buf = pool_small.tile([128, MT * NT2], f32, name="rs_buf", tag="rs_buf")
            nc.vector.memset(rs_buf[:], 1.0)
            pos_all = pool_small.tile([128, MT], f32, name="pos_all", tag="pos_all")
            nc.vector.memset(pos_all[:], 0.0)

            sim_tiles = {}

            def mm_group(ps, rhs8, m):
                for h in range(2):
                    for kk in range(KP):
                        nc.tensor.matmul(
                            ps[:, h * 512 : (h + 1) * 512],
                            own_nrm[kk][:, :, m * 128 : (m + 1) * 128],
                            rhs8[kk][h],
                            start=(kk == 0),
                            stop=(kk == KP - 1),
                            perf_mode=DR,
                        )

            def do_tile(np2):
                if np2 == 0:
                    rhs8 = [
                        [own_nrm[kk][:, :, h * 512 : (h + 1) * 512] for h in range(2)]
                        for kk in range(KP)
                    ]
                else:
                    rhs8 = []
                    for kk in range(KP):
                        row = []
                        for h in range(2):
                            raw = pool_raw.tile(
                                [128, 2, 512], bf16, name="raw", tag="raw"
                            )
                            nc.sync.dma_start(
                                out=raw[:],
                                in_=pair_src(kk, np2 * BLK + h * 512, 512),
                            )
                            r8 = pool_rhs8.tile(
                                [128, 2, 512], f8, name="r8", tag="r8"
                            )
                            if np2 in BRIDGE:
                                nc.vector.tensor_copy(r8[:], raw[:])
                            else:
                                for j in range(2):
                                    nc.vector.tensor_mul(
                                        r8[:, j, :],
                                        raw[:, j, :],
                                        binv_rem[np2][:, h * 512 : (h + 1) * 512],
                                    )
                            row.append(r8[:])
                        rhs8.append(row)
                for m in range(MT):
                    ps = psum2.tile([128, 2 * 512], f32, name="ps2", tag="ps2")
                    mm_group(ps, rhs8, m)
                    if np2 == 0:
                        # diagonal block: exp, zero self-sim, reduce on DVE
                        e = pool_e.tile([128, 1024], bf16, name="e", tag="e")
                        nc.scalar.activation(e[:], ps[:], AF.Exp, scale=PSCALE)
                        nc.vector.tensor_mul(
                            e[:, m * 128 : (m + 1) * 128],
                            e[:, m * 128 : (m + 1) * 128],
                            maskinv_sb[:],
                        )
                        nc.vector.reduce_sum(
                            out=rs_buf[:, m * NT2 : m * NT2 + 1], in_=e[:], axis=AX.X
                        )
                    elif np2 in BRIDGE:
                        sb = pool_sim.tile([128, 1024], bf16, name="sim", tag="sim")
                        nc.vector.tensor_copy(sb[:], ps[:])
                        sim_tiles[(np2, m)] = sb
                    else:
                        e = pool_e.tile([128, 1024], bf16, name="e", tag="e")
                        nc.scalar.activation(
                            e[:],
                            ps[:],
                            AF.Exp,
                            scale=PSCALE,
                            accum_out=rs_buf[:, m * NT2 + np2 : m * NT2 + np2 + 1],
                        )
                        if np2 == 4:
                            # positives: diagonal of this 128x128 slab (raw sim)
                            junk = pool_junk.tile(
                                [128, 128], f32, name="junk", tag="junk"
                            )
                            nc.vector.tensor_mul(
                                junk[:], ps[:, m * 128 : (m + 1) * 128], ident_sb[:]
                            )
                            nc.vector.reduce_sum(
                                out=pos_all[:, m : m + 1], in_=junk[:], axis=AX.X
                            )

            # own + bridge tiles first (no AllGather dependency anywhere)
            max_np2 = int(os.environ.get("CL_MAXNP2", NT2))
            do_tile(0)
            for np2 in sorted(BRIDGE):
                if np2 < max_np2:
                    do_tile(np2)

            # --- post-AllGather: remote-block inverse-norm broadcasts ---
            g_inv = pool_small.tile([8, BLK], f32, name="g_inv", tag="g_inv")
            nc.sync.dma_start(out=g_inv[:], in_=inv_out[:])
            g_inv_r = pool_small.tile([8, BLK], bf16, name="g_inv_r", tag="g_inv_r")
            nc.vector.tensor_copy(g_inv_r[:], g_inv[:])
            binv_rem = {}
            for np2 in list(sorted(BRIDGE)) + [
                j for j in range(1, NT2) if j not in BRIDGE
            ]:
                t = pool_binv.tile([128, BLK], bf16, name="binv_rem", tag="binv_rem")
                for h in range(2):
                    pr = psum1.tile([128, 512], f32, name="ps1", tag="ps1")
                    nc.tensor.matmul(
                        pr[:],
                        bsel_sb[:, np2 * 128 : (np2 + 1) * 128],
                        g_inv_r[:, h * 512 : (h + 1) * 512],
                        start=True,
                        stop=True,
                    )
                    nc.vector.tensor_copy(t[:, h * 512 : (h + 1) * 512], pr[:])
                binv_rem[np2] = t

            # remaining tiles (rhs normalized pre-matmul)
            for np2 in range(1, NT2):
                if np2 not in BRIDGE and np2 < max_np2:
                    do_tile(np2)

            # deferred bridge tiles: scale stashed raw sims, then exp
            for np2 in sorted(BRIDGE):
                if np2 >= max_np2:
                    continue
                for m in range(MT):
                    sb = sim_tiles[(np2, m)]
                    sm = pool_e.tile([128, 1024], bf16, name="e", tag="e")
                    nc.vector.tensor_mul(sm[:], sb[:], binv_rem[np2][:])
                    e = pool_e.tile([128, 1024], bf16, name="e", tag="e")
                    nc.scalar.activation(
                        e[:],
                        sm[:],
                        AF.Exp,
                        scale=PSCALE,
                        accum_out=rs_buf[:, m * NT2 + np2 : m * NT2 + np2 + 1],
                    )

            # --- logsumexp + loss ---
            rs_all = pool_small.tile([128, MT], f32, name="rs_all", tag="rs_all")
            for m in range(MT):
                nc.vector.reduce_sum(
                    out=rs_all[:, m : m + 1],
                    in_=rs_buf[:, m * NT2 : (m + 1) * NT2],
                    axis=AX.X,
                )
            lse = pool_small.tile([128, MT], f32, name="lse", tag="lse")
            nc.scalar.activation(lse[:], rs_all[:], AF.Ln)
            poss = pool_small.tile([128, MT], f32, name="poss", tag="poss")
            nc.vector.tensor_scalar_mul(poss[:], pos_all[:], PSCALE)
            diff = pool_small.tile([128, MT], f32, name="diff", tag="diff")
            nc.vector.tensor_sub(diff[:], lse[:], poss[:])
            dsum = pool_small.tile([128, 1], f32, name="dsum", tag="dsum")
            nc.vector.reduce_sum(out=dsum[:], in_=diff[:], axis=AX.X)
            pf = psum1.tile([128, 512], f32, name="ps1", tag="ps1")
            nc.tensor.matmul(pf[0:1, 0:1], dsum[:], ones_f[:], start=True, stop=True)
            part_sb = pool_small.tile([1, 1], f32, name="part_sb", tag="part_sb")
            nc.vector.tensor_copy(part_sb[:], pf[0:1, 0:1])
            nc.sync.dma_start(out=part_in[:], in_=part_sb[:])
            nc.gpsimd.collective_compute(
                "AllGather",
                mybir.AluOpType.bypass,
                ins=[part_in.opt()],
                outs=[part_out.opt()],
                replica_groups=[list(range(N_CORES))],
            )
            back = pool_small.tile([1, 8], f32, name="back", tag="back")
            nc.sync.dma_start(
                out=back[:], in_=part_out[:].rearrange("a b -> (a b)")[None, :]
            )
            tot = pool_small.tile([1, 1], f32, name="tot", tag="tot")
            nc.vector.reduce_sum(out=tot[:], in_=back[:], axis=AX.X)
            lout = pool_small.tile([1, 1], f32, name="lout", tag="lout")
            nc.scalar.mul(lout[:], tot[:], 1.0 / TWO_B)
            nc.sync.dma_start(out=loss[:], in_=lout[:])

    nc.compile()
    return nc


def make_in_maps(features_1: np.ndarray, features_2: np.ndarray):
    import ml_dtypes

    f1 = np.asarray(features_1, dtype=np.float32)
    f2 = np.asarray(features_2, dtype=np.float32)
    f = np.concatenate([f1, f2], axis=0)  # [2B, D]
    ftb = (
        np.ascontiguousarray(f.T).astype(ml_dtypes.bfloat16).reshape(D, N_CORES, BLK)
    )

    ident = np.eye(128, dtype=np.float32)
    maskinv = (1.0 - ident).astype(ml_dtypes.bfloat16)
    warm = np.zeros((1, 1), dtype=np.float32)

    in_maps = []
    for c in range(N_CORES):
        order = [(c + j) % N_CORES for j in range(N_CORES)]
        ft_c = np.ascontiguousarray(ftb[:, order, :]).reshape(D, TWO_B)
        perm_c = np.zeros((8, 8), dtype=np.float32)
        for j in range(N_CORES):
            perm_c[(c + j) % N_CORES, j] = 1.0
        bsel_c = np.repeat(perm_c, 128, axis=1).astype(ml_dtypes.bfloat16)
        in_maps.append(
            {
                "ft": ft_c,
                "bsel": bsel_c,
                "ident": ident,
                "maskinv": maskinv,
                "warm": warm,
            }
        )
    return in_maps


def kernel(features_1: np.ndarray, features_2: np.ndarray) -> np.ndarray:
    from concourse.bass_utils import run_bass_kernel_spmd

    if "nc" not in _cache:
        _cache["nc"] = _build()
    nc = _cache["nc"]

    in_maps = make_in_maps(features_1, features_2)
    res = run_bass_kernel_spmd(nc, in_maps, list(range(N_CORES)))
    out = res.results[0]["loss"]
    return np.float32(out.reshape(()))


# revision 9
# speedup vs baseline: 1.4952x; 1.0097x over previous
"""Contrastive loss (NT-Xent) on 8 Trainium2 NeuronCores.

Row-parallel over the [2B, 2B] similarity matrix: core c computes rows
[c*1024, (c+1)*1024). Inputs are passed host-transposed ([D, 2B]) in bf16 with
the column blocks rotated per core so diagonal / positive blocks land at fixed
tile indices on every core (uniform SPMD program).

Matmuls run in fp8e4 DoubleRow mode (2 MACs/cell/cycle) on RAW features:
contraction pairs are packed as [128, 2, X] tiles. Neither normalization gates
the PE — the row-side 1/norm folds into the ACT exp's per-partition scale
vector, and the column-side 1/norm either pre-scales the rhs (late tiles) or
is applied after the matmul to SBUF-stashed raw sims ("bridge" tiles that run
while the inverse-norm AllGather and its one-time CC barrier are in flight;
the column scale commutes out of the matmul). Row-wise logsumexp uses the ACT
engine's fused accumulate over 2-PSUM-bank [128,1024] tiles. Per-core partial
sums are combined with a scalar AllGather.
"""

import os
import sys

for _p in ("/opt/trn_rl_repo", "/root/.axon_site/_ro/trn_rl_repo"):
    if os.path.isdir(_p) and _p not in sys.path:
        sys.path.append(_p)

import numpy as np

B = 4096
D = 1024
TWO_B = 2 * B
TEMP = 0.07
N_CORES = 8
BLK = TWO_B // N_CORES  # 1024 columns per block
KP = D // 256  # 4 contraction pair-chunks of 2x128
NT2 = N_CORES  # 8 double-column tiles of 1024 == rotated block index
MT = BLK // 128  # 8 row slabs of 128
FSC = 8.0  # column-side pre-scale folded into inv norms (fp8 range use)
RSCALE = 1.0 / (FSC * TEMP)  # row-side inv-norm * this = exp scale

# Rotated blocks 1..N_BRIDGE run as raw-matmul "bridge" tiles while the
# inv-norm AllGather is in flight (block 0 always does). Block 4 holds the
# positives; when bridged they are extracted from the stashed raw sims.
N_BRIDGE = int(os.environ.get("CL_BRIDGE", 4))
BRIDGE = set(range(1, 1 + N_BRIDGE))

_cache = {}


def _build():
    import concourse.bass as bass
    import concourse.bacc as bacc
    import concourse.mybir as mybir
    from concourse.tile import TileContext

    f32 = mybir.dt.float32
    bf16 = mybir.dt.bfloat16
    f8 = mybir.dt.float8e4
    AF = mybir.ActivationFunctionType
    ALU = mybir.AluOpType
    AX = mybir.AxisListType
    DR = mybir.MatmulPerfMode.DoubleRow

    nc = bacc.Bacc(None, target_bir_lowering=False, debug=False)
    ft = nc.dram_tensor("ft", [D, TWO_B], bf16, kind="ExternalInput")
    bsel = nc.dram_tensor("bsel", [8, 8 * 128], bf16, kind="ExternalInput")
    ident = nc.dram_tensor("ident", [128, 128], bf16, kind="ExternalInput")
    maskinv = nc.dram_tensor("maskinv", [128, 128], bf16, kind="ExternalInput")
    warm = nc.dram_tensor("warm", [1, 1], f32, kind="ExternalInput")
    loss = nc.dram_tensor("loss", [1, 1], f32, kind="ExternalOutput")

    def pair_src(kk, c0, w):
        # DRAM view [128, 2, w]: [p, j, c] = ft[256*kk + 128*j + p, c0 + c]
        return ft[256 * kk : 256 * (kk + 1), c0 : c0 + w].rearrange(
            "(j p) c -> p j c", j=2
        )

    with TileContext(nc) as tc:
        with (
            tc.tile_pool(name="own", bufs=KP) as pool_own,
            tc.tile_pool(name="own8", bufs=KP) as pool_own8,
            tc.tile_pool(name="sq", bufs=2) as pool_sq,
            tc.tile_pool(name="binv", bufs=8) as pool_binv,
            tc.tile_pool(name="raw", bufs=12) as pool_raw,
            tc.tile_pool(name="rhs8", bufs=16) as pool_rhs8,
            tc.tile_pool(
                name="sim", bufs=max(8 * (len(BRIDGE) + 1), 1)
            ) as pool_sim,
            tc.tile_pool(name="exp", bufs=5) as pool_e,
            tc.tile_pool(name="small", bufs=1) as pool_small,
            tc.tile_pool(name="junk", bufs=2) as pool_junk,
            tc.tile_pool(name="ps2", bufs=3, space="PSUM") as psum2,
            tc.tile_pool(name="ps1", bufs=2, space="PSUM") as psum1,
            tc.tile_pool(name="dram", bufs=4, space="DRAM") as dram,
        ):
            warm_in = dram.tile([1, 1], f32, name="warm_in")
            warm_out = dram.tile([8, 1], f32, name="warm_out")
            nrm_d = dram.tile([1, BLK], f32, name="nrm_d")
            inv_in = dram.tile([1, BLK], f32, name="inv_in")
            inv_out = dram.tile([8, BLK], f32, name="inv_out")
            part_in = dram.tile([1, 1], f32, name="part_in")
            part_out = dram.tile([8, 1], f32, name="part_out")

            # --- collective-stack warmup: one DRAM->DRAM DMA dep only, fires
            # immediately so the one-time CC barrier/channel setup overlaps
            # the prologue ---
            nc.sync.dma_start(out=warm_in[:], in_=warm[0:1, 0:1])
            nc.gpsimd.collective_compute(
                "AllGather",
                mybir.AluOpType.bypass,
                ins=[warm_in.opt()],
                outs=[warm_out.opt()],
                replica_groups=[list(range(N_CORES))],
            )

            # --- constants ---
            ones_f = pool_small.tile([128, 1], f32, name="ones_f", tag="ones_f")
            nc.vector.memset(ones_f[:], 1.0)
            ones_r = pool_small.tile([128, 1], bf16, name="ones_r", tag="ones_r")
            nc.vector.tensor_copy(ones_r[:], ones_f[:])
            ident_sb = pool_small.tile([128, 128], bf16, name="ident", tag="ident")
            nc.sync.dma_start(out=ident_sb[:], in_=ident[:])
            maskinv_sb = pool_small.tile([128, 128], bf16, name="maskinv", tag="maskinv")
            nc.sync.dma_start(out=maskinv_sb[:], in_=maskinv[:])
            bsel_sb = pool_small.tile([8, 8 * 128], bf16, name="bsel", tag="bsel")
            nc.sync.dma_start(out=bsel_sb[:], in_=bsel[:])

            # --- own block: load pair tiles, cast to raw fp8 (the stationary
            # operand for every matmul; also the rhs for tile 0) ---
            own_raw = []
            own8 = []
            for kk in range(KP):
                t = pool_own.tile([128, 2, BLK], bf16, name="own_raw", tag="own_raw")
                nc.sync.dma_start(out=t[:], in_=pair_src(kk, 0, BLK))
                own_raw.append(t)
                t8 = pool_own8.tile([128, 2, BLK], f8, name="own8", tag="own8")
                nc.vector.tensor_copy(t8[:], t[:])
                own8.append(t8)

            # --- row norms (DVE ops emitted now; the norm matmuls are emitted
            # after tile 0 below so the PE starts on real work first) ---
            sq = []
            for kk in range(KP):
                s = pool_sq.tile([128, 2, BLK], bf16, name="sq", tag="sq")
                nc.vector.tensor_mul(s[:], own_raw[kk][:], own_raw[kk][:])
                sq.append(s)

            # --- accumulators ---
            rs_buf = pool_small.tile([128, MT * NT2], f32, name="rs_buf", tag="rs_buf")
            nc.vector.memset(rs_buf[:], 1.0)
            pos_all = pool_small.tile([128, MT], f32, name="pos_all", tag="pos_all")
            nc.vector.memset(pos_all[:], 0.0)

            sim_tiles = {}
            binv_rem = {}
            rowexp = {}  # filled after norms: [128, MT] exp-scale AP holder

            def mm_group(ps, rhs8, m):
                for h in range(2):
                    for kk in range(KP):
                        nc.tensor.matmul(
                            ps[:, h * 512 : (h + 1) * 512],
                            own8[kk][:, :, m * 128 : (m + 1) * 128],
                            rhs8[kk][h],
                            start=(kk == 0),
                            stop=(kk == KP - 1),
                            perf_mode=DR,
                        )

            max_np2 = int(os.environ.get("CL_MAXNP2", NT2))

            def do_tile(np2):
                bridged = np2 == 0 or np2 in BRIDGE
                if np2 == 0:
                    rhs8 = [
                        [own8[kk][:, :, h * 512 : (h + 1) * 512] for h in range(2)]
                        for kk in range(KP)
                    ]
                else:
                    rhs8 = []
                    for kk in range(KP):
                        row = []
                        for h in range(2):
                            raw = pool_raw.tile(
                                [128, 2, 512], bf16, name="raw", tag="raw"
                            )
                            nc.sync.dma_start(
                                out=raw[:],
                                in_=pair_src(kk, np2 * BLK + h * 512, 512),
                            )
                            r8 = pool_rhs8.tile(
                                [128, 2, 512], f8, name="r8", tag="r8"
                            )
                            if bridged:
                                nc.vector.tensor_copy(r8[:], raw[:])
                            else:
                                for j in range(2):
                                    nc.vector.tensor_mul(
                                        r8[:, j, :],
                                        raw[:, j, :],
                                        binv_rem[np2][:, h * 512 : (h + 1) * 512],
                                    )
                            row.append(r8[:])
                        rhs8.append(row)
                for m in range(MT):
                    ps = psum2.tile([128, 2 * 512], f32, name="ps2", tag="ps2")
                    mm_group(ps, rhs8, m)
                    if bridged:
                        sb = pool_sim.tile([128, 1024], bf16, name="sim", tag="sim")
                        nc.vector.tensor_copy(sb[:], ps[:])
                        sim_tiles[(np2, m)] = sb
                    else:
                        e = pool_e.tile([128, 1024], bf16, name="e", tag="e")
                        nc.scalar.activation(
                            e[:],
                            ps[:],
                            AF.Exp,
                            scale=rowexp["t"][:, m : m + 1],
                            accum_out=rs_buf[:, m * NT2 + np2 : m * NT2 + np2 + 1],
                        )
                        if np2 == 4:
                            # positives: diagonal of this 128x128 slab (raw sim)
                            junk = pool_junk.tile(
                                [128, 128], f32, name="junk", tag="junk"
                            )
                            nc.vector.tensor_mul(
                                junk[:], ps[:, m * 128 : (m + 1) * 128], ident_sb[:]
                            )
                            nc.vector.reduce_sum(
                                out=pos_all[:, m : m + 1], in_=junk[:], axis=AX.X
                            )

            def do_deferred(np2):
                # bridge tiles: column scale on stashed raw sims, then exp
                for m in range(MT):
                    sb = sim_tiles[(np2, m)]
                    sm = pool_e.tile([128, 1024], bf16, name="e", tag="e")
                    nc.vector.tensor_mul(sm[:], sb[:], binv_rem[np2][:])
                    if np2 == 0:
                        e = pool_e.tile([128, 1024], bf16, name="e", tag="e")
                        nc.scalar.activation(
                            e[:], sm[:], AF.Exp, scale=rowexp["t"][:, m : m + 1]
                        )
                        nc.vector.tensor_mul(
                            e[:, m * 128 : (m + 1) * 128],
                            e[:, m * 128 : (m + 1) * 128],
                            maskinv_sb[:],
                        )
                        nc.vector.reduce_sum(
                            out=rs_buf[:, m * NT2 : m * NT2 + 1], in_=e[:], axis=AX.X
                        )
                    else:
                        e = pool_e.tile([128, 1024], bf16, name="e", tag="e")
                        nc.scalar.activation(
                            e[:],
                            sm[:],
                            AF.Exp,
                            scale=rowexp["t"][:, m : m + 1],
                            accum_out=rs_buf[:, m * NT2 + np2 : m * NT2 + np2 + 1],
                        )
                        if np2 == 4:
                            junk = pool_junk.tile(
                                [128, 128], bf16, name="junkb", tag="junkb"
                            )
                            nc.vector.tensor_mul(
                                junk[:], sm[:, m * 128 : (m + 1) * 128], ident_sb[:]
                            )
                            nc.vector.reduce_sum(
                                out=pos_all[:, m : m + 1], in_=junk[:], axis=AX.X
                            )

                # tile 0: PE starts here, needs only the fp8 casts
            do_tile(0)

            # --- norm matmuls + inverse norms in [128, MT] partition layout
            # (DMA round-trip transpose keeps the DVE reciprocal off the
            # single-lane path), then the inv-norm AllGather ---
            pss = [
                psum1.tile([128, 512], f32, name="ps1", tag="ps1") for _ in range(2)
            ]
            for kk in range(KP):
                for j in range(2):
                    for h in range(2):
                        nc.tensor.matmul(
                            pss[h][0:1, :],
                            ones_r[:],
                            sq[kk][:, j, h * 512 : (h + 1) * 512],
                            start=(kk == 0 and j == 0),
                            stop=(kk == KP - 1 and j == 1),
                        )
            nrm = pool_small.tile([1, BLK], f32, name="nrm", tag="nrm")
            for h in range(2):
                # sqrt(ss/64) = norm/8  ->  reciprocal = 8/norm
                nc.scalar.activation(
                    nrm[:, h * 512 : (h + 1) * 512],
                    pss[h][0:1, :],
                    AF.Sqrt,
                    scale=1.0 / (FSC * FSC),
                )
            nc.sync.dma_start(out=nrm_d[:], in_=nrm[:])
            nrmT = pool_small.tile([128, MT], f32, name="nrmT", tag="nrmT")
            nc.sync.dma_start(
                out=nrmT[:],
                in_=nrm_d[:].rearrange("a (m p) -> p (a m)", p=128),
            )
            invT = pool_small.tile([128, MT], f32, name="invT", tag="invT")
            nc.vector.reciprocal(invT[:], nrmT[:])  # = 8/norm, [p, m] layout
            rower = pool_small.tile([128, MT], f32, name="rowexp", tag="rowexp")
            # exp scale = inv_row/(8*T) = invT/(64*T)... see RSCALE/FSC algebra
            nc.vector.tensor_scalar_mul(rower[:], invT[:], RSCALE / FSC)
            rowexp["t"] = rower
            # free-axis layout for the AllGather
            nc.sync.dma_start(
                out=inv_in[:].rearrange("a (m p) -> p (a m)", p=128), in_=invT[:]
            )
            nc.gpsimd.collective_compute(
                "AllGather",
                mybir.AluOpType.bypass,
                ins=[inv_in.opt()],
                outs=[inv_out.opt()],
                replica_groups=[list(range(N_CORES))],
            )

            # --- bridge tiles: matmul raw columns while the AllGather runs ---
            for np2 in sorted(BRIDGE):
                if np2 < max_np2:
                    do_tile(np2)

            # --- post-AllGather: per-block column inverse-norm broadcasts ---
            g_inv = pool_small.tile([8, BLK], f32, name="g_inv", tag="g_inv")
            nc.sync.dma_start(out=g_inv[:], in_=inv_out[:])
            g_inv_r = pool_small.tile([8, BLK], bf16, name="g_inv_r", tag="g_inv_r")
            nc.vector.tensor_copy(g_inv_r[:], g_inv[:])
            for np2 in [0] + sorted(BRIDGE) + [
                j for j in range(1, NT2) if j not in BRIDGE
            ]:
                t = pool_binv.tile([128, BLK], bf16, name="binv_rem", tag="binv_rem")
                for h in range(2):
                    pr = psum1.tile([128, 512], f32, name="ps1", tag="ps1")
                    nc.tensor.matmul(
                        pr[:],
                        bsel_sb[:, np2 * 128 : (np2 + 1) * 128],
                        g_inv_r[:, h * 512 : (h + 1) * 512],
                        start=True,
                        stop=True,
                    )
                    nc.vector.tensor_copy(t[:, h * 512 : (h + 1) * 512], pr[:])
                binv_rem[np2] = t

            # deferred bridge work overlaps the remaining A-path tiles
            do_deferred(0)
            for np2 in sorted(BRIDGE):
                if np2 < max_np2:
                    do_deferred(np2)

            # remaining tiles (rhs normalized pre-matmul)
            for np2 in range(1, NT2):
                if np2 not in BRIDGE and np2 < max_np2:
                    do_tile(np2)

            # --- logsumexp + loss ---
            rs_all = pool_small.tile([128, MT], f32, name="rs_all", tag="rs_all")
            for m in range(MT):
                nc.vector.reduce_sum(
                    out=rs_all[:, m : m + 1],
                    in_=rs_buf[:, m * NT2 : (m + 1) * NT2],
                    axis=AX.X,
                )
            lse = pool_small.tile([128, MT], f32, name="lse", tag="lse")
            nc.scalar.activation(lse[:], rs_all[:], AF.Ln)
            poss = pool_small.tile([128, MT], f32, name="poss", tag="poss")
            nc.vector.tensor_mul(poss[:], pos_all[:], rower[:])
            diff = pool_small.tile([128, MT], f32, name="diff", tag="diff")
            nc.vector.tensor_sub(diff[:], lse[:], poss[:])
            dsum = pool_small.tile([128, 1], f32, name="dsum", tag="dsum")
            nc.vector.reduce_sum(out=dsum[:], in_=diff[:], axis=AX.X)
            pf = psum1.tile([128, 512], f32, name="ps1", tag="ps1")
            nc.tensor.matmul(pf[0:1, 0:1], dsum[:], ones_f[:], start=True, stop=True)
            part_sb = pool_small.tile([1, 1], f32, name="part_sb", tag="part_sb")
            nc.vector.tensor_copy(part_sb[:], pf[0:1, 0:1])
            nc.sync.dma_start(out=part_in[:], in_=part_sb[:])
            nc.gpsimd.collective_compute(
                "AllGather",
                mybir.AluOpType.bypass,
                ins=[part_in.opt()],
                outs=[part_out.opt()],
                replica_groups=[list(range(N_CORES))],
            )
            back = pool_small.tile([1, 8], f32, name="back", tag="back")
            nc.sync.dma_start(
                out=back[:], in_=part_out[:].rearrange("a b -> (a b)")[None, :]
            )
            tot = pool_small.tile([1, 1], f32, name="tot", tag="tot")
            nc.vector.reduce_sum(out=tot[:], in_=back[:], axis=AX.X)
            lout = pool_small.tile([1, 1], f32, name="lout", tag="lout")
            nc.scalar.mul(lout[:], tot[:], 1.0 / TWO_B)
            nc.sync.dma_start(out=loss[:], in_=lout[:])

    nc.compile()
    return nc


def make_in_maps(features_1: np.ndarray, features_2: np.ndarray):
    import ml_dtypes

    f1 = np.asarray(features_1, dtype=np.float32)
    f2 = np.asarray(features_2, dtype=np.float32)
    f = np.concatenate([f1, f2], axis=0)  # [2B, D]
    ftb = (
        np.ascontiguousarray(f.T).astype(ml_dtypes.bfloat16).reshape(D, N_CORES, BLK)
    )

    ident = np.eye(128, dtype=np.float32).astype(ml_dtypes.bfloat16)
    maskinv = (1.0 - np.eye(128, dtype=np.float32)).astype(ml_dtypes.bfloat16)
    warm = np.zeros((1, 1), dtype=np.float32)

    in_maps = []
    for c in range(N_CORES):
        order = [(c + j) % N_CORES for j in range(N_CORES)]
        ft_c = np.ascontiguousarray(ftb[:, order, :]).reshape(D, TWO_B)
        perm_c = np.zeros((8, 8), dtype=np.float32)
        for j in range(N_CORES):
            perm_c[(c + j) % N_CORES, j] = 1.0
        bsel_c = np.repeat(perm_c, 128, axis=1).astype(ml_dtypes.bfloat16)
        in_maps.append(
            {
                "ft": ft_c,
                "bsel": bsel_c,
                "ident": ident,
                "maskinv": maskinv,
                "warm": warm,
            }
        )
    return in_maps


def kernel(features_1: np.ndarray, features_2: np.ndarray) -> np.ndarray:
    from concourse.bass_utils import run_bass_kernel_spmd

    if "nc" not in _cache:
        _cache["nc"] = _build()
    nc = _cache["nc"]

    in_maps = make_in_maps(features_1, features_2)
    res = run_bass_kernel_spmd(nc, in_maps, list(range(N_CORES)))
    out = res.results[0]["loss"]
    return np.float32(out.reshape(()))
